# revision 12
# baseline (speedup 1.0000x reference)
"""Multi-head attention (B=2, S=4096, D=1024, H=16) on 8 NeuronCores.

Sharding: core c = (batch b = c // 4, head-group g = c % 4).  Each head-group
owns 4 heads = 256 projection features.  Per core:
  - PE-transpose q/k/v[b] tiles into [d, s] layout; projections with
    row-sliced wq/wk/wv; qpT/kpT stored bf16, vp stored fp32.
  - attention per head-PAIR: scores for heads (2et, 2et+1) computed as two
    row-tiled bf16 matmuls (K=64 each) running concurrently on the PE array
    (tile_position (0,0) / (64,0)).  exp for head A on the ACT engine
    (exact), head B on the DVE via the Schraudolph bit-trick (one
    tensor_scalar, int32 out, bitcast back to f32).  Softmax normalization
    deferred via a ones column in the PV stationary operand.
  - per q-block of 512: normalize (transpose -> 1/rowsum -> transpose back,
    bf16) and project with w0 immediately, so output DMA overlaps attention.
Host sums the 4 partials per batch.
"""

import numpy as np
from contextlib import ExitStack

import concourse.bass as bass
import concourse.bacc as bacc
import concourse.tile as tile
from concourse import mybir, bass_utils
from concourse.masks import make_identity

B, S, D, H = 2, 4096, 1024, 16
DK = D // H          # 64
NCORES = 8
GROUPS = 4           # head-groups (tensor parallel)
HG = H // GROUPS     # 4 heads per group
E = HG * DK          # 256 features per group

F32 = mybir.dt.float32
F32R = mybir.dt.float32r
FP16 = mybir.dt.float16
I16 = mybir.dt.int16

P = 128              # partitions
SC = S // P          # 32 s-chunks of 128
SG = 8               # s-groups in phase T
SGW = S // SG        # 512
DC = D // P          # 8 d-chunks
QB = 512             # q-block in attention
NQB = S // QB        # 8
NST = SC             # 32 k-stripes of 128
VW = DK + 1          # vp columns per head incl. ones column (65)
VPAD = 66            # padded per-head stride in vps tile

# Schraudolph fast-exp in fp16: exp(0.125*x) ~= bitcast_fp16(int16(A*x + B))
# A = 2^10/ln2 * 0.125, B = 15*2^10 - 59.3 (min max-rel-err shift)
SCH_A = (1024.0 / 0.6931471805599453) * 0.125
SCH_B = 15300.7

EXPF = mybir.ActivationFunctionType.Exp


def _r(ap):
    return ap.bitcast(F32R)


def kernel_body(tc, q, k, v, wq, wk, wv, w0, out):
    nc = tc.nc
    ctx = ExitStack()
    with ctx:
        ident_pool = ctx.enter_context(tc.tile_pool(name="ident", bufs=1))
        identity = ident_pool.tile([P, P], F32, tag="ident", name="identity")
        make_identity(nc, identity)
        ident_bf = ident_pool.tile([P, P], FP16, tag="identbf",
                                   name="ident_bf")
        nc.vector.tensor_copy(out=ident_bf, in_=identity)

        # persistent across T..W
        w0T_pool = ctx.enter_context(tc.tile_pool(name="w0T", bufs=1))
        w0T = w0T_pool.tile([P, 2, D], FP16, tag="w0T", name="w0T")

        # persistent through phase A
        proj_pool = ctx.enter_context(tc.tile_pool(name="proj", bufs=1))
        qpT = [proj_pool.tile([P, S], FP16, tag=f"qpT{i}", name=f"qpT{i}")
               for i in range(2)]
        kpT = [proj_pool.tile([P, S], FP16, tag=f"kpT{i}", name=f"kpT{i}")
               for i in range(2)]
        vps = proj_pool.tile([P, SC, HG * VPAD], FP16, tag="vps", name="vps")
        # prefill so the per-head ones columns (offset DK in each VPAD
        # stride) are already 1.0; data copies below overwrite cols 0..DK-1
        nc.vector.memset(vps, 1.0)

        # ================= phase T: transposes + projections =================
        with tc.tile_pool(name="t_wT", bufs=1) as wT_pool, \
             tc.tile_pool(name="t_nat", bufs=3) as nat_pool, \
             tc.tile_pool(name="t_xtg", bufs=2) as xtg_pool, \
             tc.tile_pool(name="t_psum", bufs=3, space="PSUM") as ppool_t, \
             tc.tile_pool(name="t_pacc", bufs=3, space="PSUM") as ppool_a:

            wqT = wT_pool.tile([P, DC, E], FP16, tag="wqT", name="wqT")
            wkT = wT_pool.tile([P, DC, E], FP16, tag="wkT", name="wkT")
            wvT = wT_pool.tile([P, DC, E], FP16, tag="wvT", name="wvT")

            # -- weights: load natural, PE-transpose into bf16 wT tiles --
            for wsrc, wdst in ((wq, wqT), (wk, wkT), (wv, wvT)):
                for er in range(E // P):          # 2 chunks of output rows
                    wn = nat_pool.tile([P, D], F32, tag="wnat", name="wnat")
                    nc.sync.dma_start(out=wn, in_=wsrc[er * P:(er + 1) * P, :])
                    for dc in range(DC):
                        pt = ppool_t.tile([P, P], F32, tag="tp", name="tp")
                        nc.tensor.transpose(pt, wn[:, dc * P:(dc + 1) * P],
                                            identity)
                        nc.vector.tensor_copy(
                            out=wdst[:, dc, er * P:(er + 1) * P], in_=pt)
            for dc in range(DC):                  # w0 [D, E] -> w0T (bf16)
                wn = nat_pool.tile([P, E], F32, tag="w0nat", name="w0nat")
                nc.sync.dma_start(out=wn, in_=w0[dc * P:(dc + 1) * P, :])
                for ec in range(E // P):
                    pt = ppool_t.tile([P, P], F32, tag="tp", name="tp")
                    nc.tensor.transpose(pt, wn[:, ec * P:(ec + 1) * P],
                                        identity)
                    nc.vector.tensor_copy(
                        out=w0T[:, ec, dc * P:(dc + 1) * P], in_=pt)

            # -- activations: per s-group of 512: transpose + project --
            for t in range(SG):
                s0 = t * SGW
                for src, kind in ((q, "q"), (k, "k"), (v, "v")):
                    xtg = xtg_pool.tile([P, DC, SGW], FP16, tag="xtg",
                                        name="xtg")
                    for sc4 in range(SGW // P):
                        xn = nat_pool.tile([P, D], F32, tag="xnat",
                                           name="xnat")
                        nc.sync.dma_start(
                            out=xn,
                            in_=src[s0 + sc4 * P:s0 + (sc4 + 1) * P, :])
                        # batch 4 transposes into one PSUM bank so the
                        # PSUM->SBUF copy is one instruction per 4 tiles
                        for dh in range(2):
                            pt = ppool_t.tile([P, 4 * P], F32, tag="tp",
                                              name="tp")
                            for di in range(4):
                                dc = dh * 4 + di
                                nc.tensor.transpose(
                                    pt[:, di * P:(di + 1) * P],
                                    xn[:, dc * P:(dc + 1) * P], identity)
                            # xtg [p, dc, s]: 4 dc-planes share one copy via
                            # a strided view (stride over dc plane = SGW)
                            dst4 = xtg[:, dh * 4:(dh + 1) * 4,
                                       sc4 * P:(sc4 + 1) * P]
                            pt4 = pt.rearrange("p (a b) -> p a b", a=4)
                            if (sc4 + dh) % 2 == 0:
                                nc.vector.tensor_copy(out=dst4, in_=pt4)
                            else:
                                nc.scalar.copy(out=dst4, in_=pt4)
                    if kind in ("q", "k"):
                        wT = wqT if kind == "q" else wkT
                        dst = qpT if kind == "q" else kpT
                        for et in range(2):
                            acc = ppool_a.tile([P, SGW], F32, tag="acc",
                                               name="acc")
                            for dc in range(DC):
                                nc.tensor.matmul(
                                    acc,
                                    wT[:, dc, et * P:(et + 1) * P],
                                    xtg[:, dc, :],
                                    start=(dc == 0), stop=(dc == DC - 1))
                            if et == 0:
                                nc.vector.tensor_copy(
                                    out=dst[et][:, s0:s0 + SGW], in_=acc)
                            else:
                                nc.scalar.copy(
                                    out=dst[et][:, s0:s0 + SGW], in_=acc)
                    else:
                        for sc4 in range(SGW // P):
                            scg = t * (SGW // P) + sc4
                            accv = ppool_a.tile([P, E], F32, tag="acc",
                                                name="acc",
                                                padded_shape=[P, SGW])
                            for dc in range(DC):
                                nc.tensor.matmul(
                                    accv,
                                    xtg[:, dc, sc4 * P:(sc4 + 1) * P],
                                    wvT[:, dc, :],
                                    start=(dc == 0), stop=(dc == DC - 1))
                            for h in range(HG):
                                nc.vector.tensor_copy(
                                    out=vps[:, scg, h * VPAD:h * VPAD + DK],
                                    in_=accv[:, h * DK:(h + 1) * DK])

        # ========= phase A + N + W, interleaved per q-block of 512 =========
        with tc.tile_pool(name="a_ps", bufs=1, space="PSUM") as psum, \
             tc.tile_pool(name="a_sb", bufs=1) as asb:
            for qb in range(NQB):
                q0 = qb * QB
                x65s = {}
                for et in range(2):
                    hA, hB = 2 * et, 2 * et + 1
                    xaccA = psum.tile([P, QB], F32, tag="xacc", bufs=2,
                                      name="xaccA")
                    xaccB = psum.tile([P, QB], F32, tag="xacc", bufs=2,
                                      name="xaccB")
                    def emit_pv(kkp, a2):
                        nc.tensor.matmul(
                            xaccA[:VW, :],
                            vps[:, kkp, hA * VPAD:hA * VPAD + VW],
                            a2[:, 0:QB],
                            start=(kkp == 0), stop=(kkp == NST - 1))
                        nc.tensor.matmul(
                            xaccB[:VW, :],
                            vps[:, kkp, hB * VPAD:hB * VPAD + VW],
                            a2[:, QB:2 * QB],
                            start=(kkp == 0), stop=(kkp == NST - 1))

                    prev = None
                    for kk in range(NST):
                        st2 = psum.tile([P, 2 * QB], F32, tag="st", bufs=3,
                                        name="st2")
                        nc.tensor.matmul(
                            st2[:, 0:QB],
                            kpT[et][0:DK, kk * P:(kk + 1) * P],
                            qpT[et][0:DK, q0:q0 + QB],
                            start=True, stop=True, tile_position=(0, 0))
                        nc.tensor.matmul(
                            st2[:, QB:2 * QB],
                            kpT[et][DK:P, kk * P:(kk + 1) * P],
                            qpT[et][DK:P, q0:q0 + QB],
                            start=True, stop=True, tile_position=(DK, 0))
                        att2 = asb.tile([P, 2 * QB], FP16, tag="att", bufs=6,
                                        name="att2")
                        if kk % 3 == 0:
                            # exact exp for both heads in ONE fused ACT
                            # instruction every 3rd stripe: Schraudolph share
                            # 1/3, and no double-exp burst on the ACT queue
                            nc.scalar.activation(att2, st2, EXPF, scale=0.125)
                        else:
                            nc.scalar.activation(att2[:, 0:QB], st2[:, 0:QB],
                                                 EXPF, scale=0.125)
                            nc.vector.tensor_scalar(
                                out=att2.bitcast(I16)[:, QB:2 * QB],
                                in0=st2[:, QB:2 * QB],
                                scalar1=SCH_A, scalar2=SCH_B,
                                op0=mybir.AluOpType.mult,
                                op1=mybir.AluOpType.add)
                        # software pipeline: PV for stripe kk-1 issues behind
                        # this stripe's scores so the PE never waits on exp
                        if prev is not None:
                            emit_pv(kk - 1, prev)
                        prev = att2
                    emit_pv(NST - 1, prev)
                    x65A = asb.tile([VW, QB], F32, tag="x65", bufs=8,
                                    name="x65A")
                    x65B = asb.tile([VW, QB], F32, tag="x65", bufs=8,
                                    name="x65B")
                    nc.vector.tensor_copy(out=x65A, in_=xaccA[:VW, :])
                    nc.vector.tensor_copy(out=x65B, in_=xaccB[:VW, :])
                    x65s[hA], x65s[hB] = x65A, x65B

                # ---- phase N for this q-block: normalize by 1/rowsum ----
                xw = {}
                for et in range(2):
                    xwt = asb.tile([P, QB], FP16, tag="xw", bufs=4, name="xw")
                    for qc in range(QB // P):
                        xs2 = asb.tile([P, 2 * DK], FP16, tag="xs2", bufs=4,
                                       name="xs2")
                        for hp in range(2):
                            h = 2 * et + hp
                            tp = psum.tile([P, QB], F32, tag="st", bufs=3,
                                           name="ntp")
                            nc.tensor.transpose(
                                tp[:, :VW],
                                x65s[h][:, qc * P:(qc + 1) * P],
                                identity[:VW, :VW])
                            rcp = asb.tile([P, 1], F32, tag="rcp", bufs=4,
                                           name="rcp")
                            nc.vector.reciprocal(rcp, tp[:, DK:DK + 1])
                            nc.vector.tensor_scalar_mul(
                                xs2[:, hp * DK:(hp + 1) * DK],
                                tp[:, 0:DK], rcp)
                        tb = psum.tile([P, QB], F32, tag="st", bufs=3,
                                       name="ntb")
                        tbb = tb.bitcast(FP16)
                        nc.tensor.transpose(tbb[:, :P], xs2, ident_bf)
                        nc.vector.tensor_copy(
                            out=xwt[:, qc * P:(qc + 1) * P], in_=tbb[:, :P])
                    xw[et] = xwt

                # ---- phase W for this q-block: output projection ----
                for qc in range(QB // P):
                    oacc = [psum.tile([P, 512], F32, tag="st", bufs=3,
                                      name=f"oacc{j}") for j in range(2)]
                    for et in range(2):
                        for j in range(2):
                            nc.tensor.matmul(
                                oacc[j],
                                xw[et][:, qc * P:(qc + 1) * P],
                                w0T[:, et, j * 512:(j + 1) * 512],
                                start=(et == 0), stop=(et == 1))
                    osb = asb.tile([P, D], F32, tag="osb", bufs=3, name="osb")
                    nc.vector.tensor_copy(out=osb[:, 0:512], in_=oacc[0])
                    nc.vector.tensor_copy(out=osb[:, 512:1024], in_=oacc[1])
                    nc.sync.dma_start(
                        out=out[q0 + qc * P:q0 + (qc + 1) * P, :], in_=osb)


def build_program():
    nc = bacc.Bacc("TRN2", target_bir_lowering=False, debug=False,
                   num_devices=NCORES)
    q = nc.dram_tensor("q", (S, D), F32, kind="ExternalInput").ap()
    k = nc.dram_tensor("k", (S, D), F32, kind="ExternalInput").ap()
    v = nc.dram_tensor("v", (S, D), F32, kind="ExternalInput").ap()
    wq = nc.dram_tensor("wq", (E, D), F32, kind="ExternalInput").ap()
    wk = nc.dram_tensor("wk", (E, D), F32, kind="ExternalInput").ap()
    wv = nc.dram_tensor("wv", (E, D), F32, kind="ExternalInput").ap()
    w0 = nc.dram_tensor("w0", (D, E), F32, kind="ExternalInput").ap()
    out = nc.dram_tensor("out", (S, D), F32, kind="ExternalOutput").ap()
    with tile.TileContext(nc) as tc:
        kernel_body(tc, q, k, v, wq, wk, wv, w0, out)
    nc.compile()
    return nc


_NC_CACHE = None


def _get_program():
    global _NC_CACHE
    if _NC_CACHE is None:
        _NC_CACHE = build_program()
    return _NC_CACHE


def make_in_maps(q, k, v, wq, wk, wv, w0):
    arrs = [np.asarray(a, dtype=np.float32)
            for a in (q, k, v, wq, wk, wv, w0)]
    q, k, v, wq, wk, wv, w0 = arrs
    in_maps = []
    for c in range(NCORES):
        b, g = c // GROUPS, c % GROUPS
        e0 = g * E
        in_maps.append({
            "q": np.ascontiguousarray(q[b]),
            "k": np.ascontiguousarray(k[b]),
            "v": np.ascontiguousarray(v[b]),
            "wq": np.ascontiguousarray(wq[e0:e0 + E, :]),
            "wk": np.ascontiguousarray(wk[e0:e0 + E, :]),
            "wv": np.ascontiguousarray(wv[e0:e0 + E, :]),
            "w0": np.ascontiguousarray(w0[:, e0:e0 + E]),
        })
    return in_maps


def gather_out(results):
    out = np.zeros((B, S, D), dtype=np.float32)
    for c in range(NCORES):
        b = c // GROUPS
        out[b] += results[c]["out"]
    return out


def _install_ntff_hook_shim():
    """This image's antenv lacks axon_hooks; recreate it so trace=True works.

    Mirrors trn_agent_boot.trn_boot._ntff_profile_via_ctypes against
    /opt/axon/libaxon_pjrt.so.
    """
    import sys, types, ctypes, contextlib
    if "antenv.axon_hooks" in sys.modules:
        return
    mod = types.ModuleType("antenv.axon_hooks")
    mod._hook = None

    def set_axon_ntff_profile_hook(h):
        mod._hook = h

    def get_axon_ntff_profile_hook():
        return mod._hook

    mod.set_axon_ntff_profile_hook = set_axon_ntff_profile_hook
    mod.get_axon_ntff_profile_hook = get_axon_ntff_profile_hook
    sys.modules["antenv.axon_hooks"] = mod
    try:
        import antenv
        antenv.axon_hooks = mod
    except ImportError:
        pass

    so_path = "/opt/axon/libaxon_pjrt.so"
    try:
        lib = ctypes.CDLL(so_path)
        if not hasattr(lib, "axon_start_nrt_profile"):
            return
        lib.axon_start_nrt_profile.argtypes = [
            ctypes.POINTER(ctypes.c_int64), ctypes.c_size_t]
        lib.axon_start_nrt_profile.restype = ctypes.c_int64
        lib.axon_stop_nrt_profile.argtypes = [ctypes.c_char_p]
        lib.axon_stop_nrt_profile.restype = ctypes.c_int64
    except OSError:
        return

    @contextlib.contextmanager
    def _hook(output_dir, device_ids):
        import jax
        jax.devices()
        if device_ids:
            ids = (ctypes.c_int64 * len(device_ids))(*device_ids)
            rc = lib.axon_start_nrt_profile(ids, len(device_ids))
        else:
            rc = lib.axon_start_nrt_profile(None, 0)
        if rc != 0:
            raise RuntimeError(f"axon_start_nrt_profile rc={rc}")
        try:
            yield
        finally:
            n = lib.axon_stop_nrt_profile(str(output_dir).encode())
            print(f"profile: {n} file(s) written to {output_dir}")

    mod._hook = _hook


def kernel(q, k, v, wq, wk, wv, w0, _trace=False, _tmpdir=None):
    if _trace:
        _install_ntff_hook_shim()
    nc = _get_program()
    in_maps = make_in_maps(q, k, v, wq, wk, wv, w0)
    res = bass_utils.run_bass_kernel_spmd(
        nc, in_maps, core_ids=list(range(NCORES)),
        trace=_trace, tmpdir=_tmpdir)
    out = gather_out(res.results)
    if _trace:
        return out, res
    return out


# revision 14
# speedup vs baseline: 1.0371x; 1.0371x over previous
"""Multi-head attention (B=2, S=4096, D=1024, H=16) on 8 NeuronCores.

Sharding: core c = (batch b = c // 4, head-group g = c % 4).  Each head-group
owns 4 heads = 256 projection features.  Per core:
  - PE-transpose q/k/v[b] tiles into [d, s] layout; projections with
    row-sliced wq/wk/wv; qpT/kpT stored bf16, vp stored fp32.
  - attention per head-PAIR: scores for heads (2et, 2et+1) computed as two
    row-tiled bf16 matmuls (K=64 each) running concurrently on the PE array
    (tile_position (0,0) / (64,0)).  exp for head A on the ACT engine
    (exact), head B on the DVE via the Schraudolph bit-trick (one
    tensor_scalar, int32 out, bitcast back to f32).  Softmax normalization
    deferred via a ones column in the PV stationary operand.
  - per q-block of 512: normalize (transpose -> 1/rowsum -> transpose back,
    bf16) and project with w0 immediately, so output DMA overlaps attention.
Host sums the 4 partials per batch.
"""

import numpy as np
from contextlib import ExitStack

import concourse.bass as bass
import concourse.bacc as bacc
import concourse.tile as tile
from concourse import mybir, bass_utils
from concourse.masks import make_identity

B, S, D, H = 2, 4096, 1024, 16
DK = D // H          # 64
NCORES = 8
GROUPS = 4           # head-groups (tensor parallel)
HG = H // GROUPS     # 4 heads per group
E = HG * DK          # 256 features per group

F32 = mybir.dt.float32
F32R = mybir.dt.float32r
FP16 = mybir.dt.float16
I16 = mybir.dt.int16

P = 128              # partitions
SC = S // P          # 32 s-chunks of 128
SG = 8               # s-groups in phase T
SGW = S // SG        # 512
DC = D // P          # 8 d-chunks
QB = 512             # q-block in attention
NQB = S // QB        # 8
NST = SC             # 32 k-stripes of 128
VW = DK + 1          # vp columns per head incl. ones column (65)
VPAD = 66            # padded per-head stride in vps tile

# Schraudolph fast-exp in fp16: exp(0.125*x) ~= bitcast_fp16(int16(A*x + B))
# A = 2^10/ln2 * 0.125, B = 15*2^10 - 59.3 (min max-rel-err shift)
SCH_A = (1024.0 / 0.6931471805599453) * 0.125
SCH_B = 15300.7

EXPF = mybir.ActivationFunctionType.Exp


def _r(ap):
    return ap.bitcast(F32R)


def kernel_body(tc, q, k, v, wq, wk, wv, w0, out):
    nc = tc.nc
    ctx = ExitStack()
    with ctx:
        ident_pool = ctx.enter_context(tc.tile_pool(name="ident", bufs=1))
        identity = ident_pool.tile([P, P], F32, tag="ident", name="identity")
        make_identity(nc, identity)
        ident_bf = ident_pool.tile([P, P], FP16, tag="identbf",
                                   name="ident_bf")
        nc.vector.tensor_copy(out=ident_bf, in_=identity)

        # persistent across T..W
        w0T_pool = ctx.enter_context(tc.tile_pool(name="w0T", bufs=1))
        w0T = w0T_pool.tile([P, 2, D], FP16, tag="w0T", name="w0T")

        # persistent through phase A
        proj_pool = ctx.enter_context(tc.tile_pool(name="proj", bufs=1))
        qpT = [proj_pool.tile([P, S], FP16, tag=f"qpT{i}", name=f"qpT{i}")
               for i in range(2)]
        kpT = [proj_pool.tile([P, S], FP16, tag=f"kpT{i}", name=f"kpT{i}")
               for i in range(2)]
        vps = proj_pool.tile([P, SC, HG * VPAD], FP16, tag="vps", name="vps")
        # prefill so the per-head ones columns (offset DK in each VPAD
        # stride) are already 1.0; data copies below overwrite cols 0..DK-1
        nc.vector.memset(vps, 1.0)

        # ================= phase T: transposes + projections =================
        with tc.tile_pool(name="t_wT", bufs=1) as wT_pool, \
             tc.tile_pool(name="t_nat", bufs=3) as nat_pool, \
             tc.tile_pool(name="t_xtg", bufs=2) as xtg_pool, \
             tc.tile_pool(name="t_psum", bufs=3, space="PSUM") as ppool_t, \
             tc.tile_pool(name="t_pacc", bufs=3, space="PSUM") as ppool_a:

            wqT = wT_pool.tile([P, DC, E], FP16, tag="wqT", name="wqT")
            wkT = wT_pool.tile([P, DC, E], FP16, tag="wkT", name="wkT")
            wvT = wT_pool.tile([P, DC, E], FP16, tag="wvT", name="wvT")

            # -- weights: load natural, PE-transpose into bf16 wT tiles --
            for wsrc, wdst in ((wq, wqT), (wk, wkT), (wv, wvT)):
                for er in range(E // P):          # 2 chunks of output rows
                    wn = nat_pool.tile([P, D], F32, tag="wnat", name="wnat")
                    nc.sync.dma_start(out=wn, in_=wsrc[er * P:(er + 1) * P, :])
                    for dc in range(DC):
                        pt = ppool_t.tile([P, P], F32, tag="tp", name="tp")
                        nc.tensor.transpose(pt, wn[:, dc * P:(dc + 1) * P],
                                            identity)
                        nc.vector.tensor_copy(
                            out=wdst[:, dc, er * P:(er + 1) * P], in_=pt)
            for dc in range(DC):                  # w0 [D, E] -> w0T (bf16)
                wn = nat_pool.tile([P, E], F32, tag="w0nat", name="w0nat")
                nc.sync.dma_start(out=wn, in_=w0[dc * P:(dc + 1) * P, :])
                for ec in range(E // P):
                    pt = ppool_t.tile([P, P], F32, tag="tp", name="tp")
                    nc.tensor.transpose(pt, wn[:, ec * P:(ec + 1) * P],
                                        identity)
                    nc.vector.tensor_copy(
                        out=w0T[:, ec, dc * P:(dc + 1) * P], in_=pt)

            # -- activations: per s-group of 512: transpose + project --
            for t in range(SG):
                s0 = t * SGW
                for src, kind in ((q, "q"), (k, "k"), (v, "v")):
                    xtg = xtg_pool.tile([P, DC, SGW], FP16, tag="xtg",
                                        name="xtg")
                    for sc4 in range(SGW // P):
                        xn = nat_pool.tile([P, D], F32, tag="xnat",
                                           name="xnat")
                        nc.sync.dma_start(
                            out=xn,
                            in_=src[s0 + sc4 * P:s0 + (sc4 + 1) * P, :])
                        # batch 4 transposes into one PSUM bank so the
                        # PSUM->SBUF copy is one instruction per 4 tiles
                        for dh in range(2):
                            pt = ppool_t.tile([P, 4 * P], F32, tag="tp",
                                              name="tp")
                            for di in range(4):
                                dc = dh * 4 + di
                                nc.tensor.transpose(
                                    pt[:, di * P:(di + 1) * P],
                                    xn[:, dc * P:(dc + 1) * P], identity)
                            # xtg [p, dc, s]: 4 dc-planes share one copy via
                            # a strided view (stride over dc plane = SGW)
                            dst4 = xtg[:, dh * 4:(dh + 1) * 4,
                                       sc4 * P:(sc4 + 1) * P]
                            pt4 = pt.rearrange("p (a b) -> p a b", a=4)
                            if (sc4 + dh) % 2 == 0:
                                nc.vector.tensor_copy(out=dst4, in_=pt4)
                            else:
                                nc.scalar.copy(out=dst4, in_=pt4)
                    if kind in ("q", "k"):
                        wT = wqT if kind == "q" else wkT
                        dst = qpT if kind == "q" else kpT
                        for et in range(2):
                            acc = ppool_a.tile([P, SGW], F32, tag="acc",
                                               name="acc")
                            for dc in range(DC):
                                nc.tensor.matmul(
                                    acc,
                                    wT[:, dc, et * P:(et + 1) * P],
                                    xtg[:, dc, :],
                                    start=(dc == 0), stop=(dc == DC - 1))
                            if et == 0:
                                nc.vector.tensor_copy(
                                    out=dst[et][:, s0:s0 + SGW], in_=acc)
                            else:
                                nc.scalar.copy(
                                    out=dst[et][:, s0:s0 + SGW], in_=acc)
                    else:
                        for sc4 in range(SGW // P):
                            scg = t * (SGW // P) + sc4
                            accv = ppool_a.tile([P, E], F32, tag="acc",
                                                name="acc",
                                                padded_shape=[P, SGW])
                            for dc in range(DC):
                                nc.tensor.matmul(
                                    accv,
                                    xtg[:, dc, sc4 * P:(sc4 + 1) * P],
                                    wvT[:, dc, :],
                                    start=(dc == 0), stop=(dc == DC - 1))
                            for h in range(HG):
                                nc.vector.tensor_copy(
                                    out=vps[:, scg, h * VPAD:h * VPAD + DK],
                                    in_=accv[:, h * DK:(h + 1) * DK])

        # ========= phase A + N + W, interleaved per q-block of 512 =========
        with tc.tile_pool(name="a_ps", bufs=1, space="PSUM") as psum, \
             tc.tile_pool(name="a_sb", bufs=1) as asb:
            for qb in range(NQB):
                q0 = qb * QB
                x65s = {}
                for et in range(2):
                    hA, hB = 2 * et, 2 * et + 1
                    xaccA = psum.tile([P, QB], F32, tag="xacc", bufs=2,
                                      name="xaccA")
                    xaccB = psum.tile([P, QB], F32, tag="xacc", bufs=2,
                                      name="xaccB")
                    def emit_pv(kkp, ab):
                        nc.tensor.matmul(
                            xaccA[:VW, :],
                            vps[:, kkp, hA * VPAD:hA * VPAD + VW],
                            ab[0],
                            start=(kkp == 0), stop=(kkp == NST - 1))
                        nc.tensor.matmul(
                            xaccB[:VW, :],
                            vps[:, kkp, hB * VPAD:hB * VPAD + VW],
                            ab[1],
                            start=(kkp == 0), stop=(kkp == NST - 1))

                    prev = None
                    for kk in range(NST):
                        stA = psum.tile([P, QB], F32, tag="st", bufs=6,
                                        name="stA")
                        stB = psum.tile([P, QB], F32, tag="st", bufs=6,
                                        name="stB")
                        nc.tensor.matmul(
                            stA,
                            kpT[et][0:DK, kk * P:(kk + 1) * P],
                            qpT[et][0:DK, q0:q0 + QB],
                            start=True, stop=True, tile_position=(0, 0))
                        nc.tensor.matmul(
                            stB,
                            kpT[et][DK:P, kk * P:(kk + 1) * P],
                            qpT[et][DK:P, q0:q0 + QB],
                            start=True, stop=True, tile_position=(DK, 0))
                        attA = asb.tile([P, QB], FP16, tag="att", bufs=6,
                                        name="attA")
                        attB = asb.tile([P, QB], FP16, tag="att", bufs=6,
                                        name="attB")
                        nc.scalar.activation(attA, stA, EXPF, scale=0.125)
                        if kk % 3 == 0:
                            # exact exp for head B every 3rd stripe: keeps the
                            # Schraudolph share at 1/3, ACT load ~= PE load
                            nc.scalar.activation(attB, stB, EXPF, scale=0.125)
                        else:
                            nc.vector.tensor_scalar(
                                out=attB.bitcast(I16), in0=stB,
                                scalar1=SCH_A, scalar2=SCH_B,
                                op0=mybir.AluOpType.mult,
                                op1=mybir.AluOpType.add)
                        # software pipeline: PV for stripe kk-1 issues behind
                        # this stripe's scores so the PE never waits on exp
                        if prev is not None:
                            emit_pv(kk - 1, prev)
                        prev = (attA, attB)
                    emit_pv(NST - 1, prev)
                    x65A = asb.tile([VW, QB], F32, tag="x65", bufs=8,
                                    name="x65A")
                    x65B = asb.tile([VW, QB], F32, tag="x65", bufs=8,
                                    name="x65B")
                    nc.scalar.copy(out=x65A, in_=xaccA[:VW, :])
                    nc.vector.tensor_copy(out=x65B, in_=xaccB[:VW, :])
                    x65s[hA], x65s[hB] = x65A, x65B

                # ---- phase N + W, two-stage pipeline over q-chunks so
                # stage-2 PE work (transpose-back + w0 matmuls) fills the
                # DVE-latency gaps of stage-1 (transpose -> 1/rowsum) ----
                def nw_stage1(qc):
                    xs2s = []
                    for et in range(2):
                        xs2 = asb.tile([P, 2 * DK], FP16, tag="xs2", bufs=6,
                                       name="xs2")
                        for hp in range(2):
                            h = 2 * et + hp
                            tp = psum.tile([P, QB], F32, tag="st", bufs=6,
                                           name="ntp")
                            nc.tensor.transpose(
                                tp[:, :VW],
                                x65s[h][:, qc * P:(qc + 1) * P],
                                identity[:VW, :VW])
                            rcp = asb.tile([P, 1], F32, tag="rcp", bufs=8,
                                           name="rcp")
                            nc.vector.reciprocal(rcp, tp[:, DK:DK + 1])
                            nc.vector.tensor_scalar_mul(
                                xs2[:, hp * DK:(hp + 1) * DK],
                                tp[:, 0:DK], rcp)
                        xs2s.append(xs2)
                    return xs2s

                def nw_stage2(qc, xs2s):
                    xwc = []
                    for et in range(2):
                        tb = psum.tile([P, QB], F32, tag="st", bufs=6,
                                       name="ntb")
                        tbb = tb.bitcast(FP16)
                        nc.tensor.transpose(tbb[:, :P], xs2s[et], ident_bf)
                        xwt = asb.tile([P, P], FP16, tag="xw", bufs=4,
                                       name="xw")
                        nc.vector.tensor_copy(out=xwt, in_=tbb[:, :P])
                        xwc.append(xwt)
                    oacc = [psum.tile([P, 512], F32, tag="st", bufs=6,
                                      name=f"oacc{j}") for j in range(2)]
                    for et in range(2):
                        for j in range(2):
                            nc.tensor.matmul(
                                oacc[j],
                                xwc[et],
                                w0T[:, et, j * 512:(j + 1) * 512],
                                start=(et == 0), stop=(et == 1))
                    osb = asb.tile([P, D], F32, tag="osb", bufs=3, name="osb")
                    nc.vector.tensor_copy(out=osb[:, 0:512], in_=oacc[0])
                    nc.scalar.copy(out=osb[:, 512:1024], in_=oacc[1])
                    nc.sync.dma_start(
                        out=out[q0 + qc * P:q0 + (qc + 1) * P, :], in_=osb)

                pend = []
                for qc in range(QB // P):
                    pend.append((qc, nw_stage1(qc)))
                    if qc >= 1:
                        nw_stage2(*pend.pop(0))
                while pend:
                    nw_stage2(*pend.pop(0))


def build_program():
    nc = bacc.Bacc("TRN2", target_bir_lowering=False, debug=False,
                   num_devices=NCORES)
    q = nc.dram_tensor("q", (S, D), F32, kind="ExternalInput").ap()
    k = nc.dram_tensor("k", (S, D), F32, kind="ExternalInput").ap()
    v = nc.dram_tensor("v", (S, D), F32, kind="ExternalInput").ap()
    wq = nc.dram_tensor("wq", (E, D), F32, kind="ExternalInput").ap()
    wk = nc.dram_tensor("wk", (E, D), F32, kind="ExternalInput").ap()
    wv = nc.dram_tensor("wv", (E, D), F32, kind="ExternalInput").ap()
    w0 = nc.dram_tensor("w0", (D, E), F32, kind="ExternalInput").ap()
    out = nc.dram_tensor("out", (S, D), F32, kind="ExternalOutput").ap()
    with tile.TileContext(nc) as tc:
        kernel_body(tc, q, k, v, wq, wk, wv, w0, out)
    nc.compile()
    return nc


_NC_CACHE = None


def _get_program():
    global _NC_CACHE
    if _NC_CACHE is None:
        _NC_CACHE = build_program()
    return _NC_CACHE


def make_in_maps(q, k, v, wq, wk, wv, w0):
    arrs = [np.asarray(a, dtype=np.float32)
            for a in (q, k, v, wq, wk, wv, w0)]
    q, k, v, wq, wk, wv, w0 = arrs
    in_maps = []
    for c in range(NCORES):
        b, g = c // GROUPS, c % GROUPS
        e0 = g * E
        in_maps.append({
            "q": np.ascontiguousarray(q[b]),
            "k": np.ascontiguousarray(k[b]),
            "v": np.ascontiguousarray(v[b]),
            "wq": np.ascontiguousarray(wq[e0:e0 + E, :]),
            "wk": np.ascontiguousarray(wk[e0:e0 + E, :]),
            "wv": np.ascontiguousarray(wv[e0:e0 + E, :]),
            "w0": np.ascontiguousarray(w0[:, e0:e0 + E]),
        })
    return in_maps


def gather_out(results):
    out = np.zeros((B, S, D), dtype=np.float32)
    for c in range(NCORES):
        b = c // GROUPS
        out[b] += results[c]["out"]
    return out


def _install_ntff_hook_shim():
    """This image's antenv lacks axon_hooks; recreate it so trace=True works.

    Mirrors trn_agent_boot.trn_boot._ntff_profile_via_ctypes against
    /opt/axon/libaxon_pjrt.so.
    """
    import sys, types, ctypes, contextlib
    if "antenv.axon_hooks" in sys.modules:
        return
    mod = types.ModuleType("antenv.axon_hooks")
    mod._hook = None

    def set_axon_ntff_profile_hook(h):
        mod._hook = h

    def get_axon_ntff_profile_hook():
        return mod._hook

    mod.set_axon_ntff_profile_hook = set_axon_ntff_profile_hook
    mod.get_axon_ntff_profile_hook = get_axon_ntff_profile_hook
    sys.modules["antenv.axon_hooks"] = mod
    try:
        import antenv
        antenv.axon_hooks = mod
    except ImportError:
        pass

    so_path = "/opt/axon/libaxon_pjrt.so"
    try:
        lib = ctypes.CDLL(so_path)
        if not hasattr(lib, "axon_start_nrt_profile"):
            return
        lib.axon_start_nrt_profile.argtypes = [
            ctypes.POINTER(ctypes.c_int64), ctypes.c_size_t]
        lib.axon_start_nrt_profile.restype = ctypes.c_int64
        lib.axon_stop_nrt_profile.argtypes = [ctypes.c_char_p]
        lib.axon_stop_nrt_profile.restype = ctypes.c_int64
    except OSError:
        return

    @contextlib.contextmanager
    def _hook(output_dir, device_ids):
        import jax
        jax.devices()
        if device_ids:
            ids = (ctypes.c_int64 * len(device_ids))(*device_ids)
            rc = lib.axon_start_nrt_profile(ids, len(device_ids))
        else:
            rc = lib.axon_start_nrt_profile(None, 0)
        if rc != 0:
            raise RuntimeError(f"axon_start_nrt_profile rc={rc}")
        try:
            yield
        finally:
            n = lib.axon_stop_nrt_profile(str(output_dir).encode())
            print(f"profile: {n} file(s) written to {output_dir}")

    mod._hook = _hook


def kernel(q, k, v, wq, wk, wv, w0, _trace=False, _tmpdir=None):
    if _trace:
        _install_ntff_hook_shim()
    nc = _get_program()
    in_maps = make_in_maps(q, k, v, wq, wk, wv, w0)
    res = bass_utils.run_bass_kernel_spmd(
        nc, in_maps, core_ids=list(range(NCORES)),
        trace=_trace, tmpdir=_tmpdir)
    out = gather_out(res.results)
    if _trace:
        return out, res
    return out


# revision 17
# speedup vs baseline: 1.0949x; 1.0558x over previous
"""Multi-head attention (B=2, S=4096, D=1024, H=16) on 8 NeuronCores.

Sharding: core c = (batch b = c // 4, head-group g = c % 4).  Each head-group
owns 4 heads = 256 projection features.  Per core:
  - PE-transpose q/k/v[b] tiles into [d, s] layout; projections with
    row-sliced wq/wk/wv; qpT/kpT stored bf16, vp stored fp32.
  - attention per head-PAIR: scores for heads (2et, 2et+1) computed as two
    row-tiled bf16 matmuls (K=64 each) running concurrently on the PE array
    (tile_position (0,0) / (64,0)).  exp for head A on the ACT engine
    (exact), head B on the DVE via the Schraudolph bit-trick (one
    tensor_scalar, int32 out, bitcast back to f32).  Softmax normalization
    deferred via a ones column in the PV stationary operand.
  - per q-block of 512: normalize (transpose -> 1/rowsum -> transpose back,
    bf16) and project with w0 immediately, so output DMA overlaps attention.
Host sums the 4 partials per batch.
"""

import numpy as np
from contextlib import ExitStack

import concourse.bass as bass
import concourse.bacc as bacc
import concourse.tile as tile
from concourse import mybir, bass_utils
from concourse.masks import make_identity

B, S, D, H = 2, 4096, 1024, 16
DK = D // H          # 64
NCORES = 8
GROUPS = 4           # head-groups (tensor parallel)
HG = H // GROUPS     # 4 heads per group
E = HG * DK          # 256 features per group

F32 = mybir.dt.float32
F32R = mybir.dt.float32r
FP16 = mybir.dt.float16
I16 = mybir.dt.int16

P = 128              # partitions
SC = S // P          # 32 s-chunks of 128
SG = 8               # s-groups in phase T
SGW = S // SG        # 512
DC = D // P          # 8 d-chunks
QB = 512             # q-block in attention
NQB = S // QB        # 8
NST = SC             # 32 k-stripes of 128
VW = DK + 1          # vp columns per head incl. ones column (65)
VPAD = 66            # padded per-head stride in vps tile

# Schraudolph fast-exp in fp16: exp(0.125*x) ~= bitcast_fp16(int16(A*x + B))
# A = 2^10/ln2 * 0.125, B = 15*2^10 - 59.3 (min max-rel-err shift)
SCH_A = (1024.0 / 0.6931471805599453) * 0.125
SCH_B = 15300.7

EXPF = mybir.ActivationFunctionType.Exp


def _r(ap):
    return ap.bitcast(F32R)


def kernel_body(tc, q, k, v, wq, wk, wv, w0, out):
    nc = tc.nc
    ctx = ExitStack()
    with ctx:
        ident_pool = ctx.enter_context(tc.tile_pool(name="ident", bufs=1))
        identity = ident_pool.tile([P, P], F32, tag="ident", name="identity")
        make_identity(nc, identity)
        ident_bf = ident_pool.tile([P, P], FP16, tag="identbf",
                                   name="ident_bf")
        nc.vector.tensor_copy(out=ident_bf, in_=identity)

        # persistent across T..W
        w0T_pool = ctx.enter_context(tc.tile_pool(name="w0T", bufs=1))
        w0T = w0T_pool.tile([P, 2, D], FP16, tag="w0T", name="w0T")

        # persistent through phase A
        proj_pool = ctx.enter_context(tc.tile_pool(name="proj", bufs=1))
        qpT = [proj_pool.tile([P, S], FP16, tag=f"qpT{i}", name=f"qpT{i}")
               for i in range(2)]
        kpT = [proj_pool.tile([P, S], FP16, tag=f"kpT{i}", name=f"kpT{i}")
               for i in range(2)]
        vps = proj_pool.tile([P, SC, HG * VPAD], FP16, tag="vps", name="vps")
        # prefill so the per-head ones columns (offset DK in each VPAD
        # stride) are already 1.0; data copies below overwrite cols 0..DK-1
        nc.vector.memset(vps, 1.0)

        # ================= phase T: transposes + projections =================
        with tc.tile_pool(name="t_wT", bufs=1) as wT_pool, \
             tc.tile_pool(name="t_nat", bufs=3) as nat_pool, \
             tc.tile_pool(name="t_xtg", bufs=2) as xtg_pool, \
             tc.tile_pool(name="t_psum", bufs=3, space="PSUM") as ppool_t, \
             tc.tile_pool(name="t_pacc", bufs=3, space="PSUM") as ppool_a:

            wqT = wT_pool.tile([P, DC, E], FP16, tag="wqT", name="wqT")
            wkT = wT_pool.tile([P, DC, E], FP16, tag="wkT", name="wkT")
            wvT = wT_pool.tile([P, DC, E], FP16, tag="wvT", name="wvT")

            # -- weights: load natural, fp16-convert, PE-transpose (1 cyc/
            # row) 8 tiles into one PSUM bank, single strided copy out --
            for wsrc, wdst in ((wq, wqT), (wk, wkT), (wv, wvT)):
                for er in range(E // P):          # 2 chunks of output rows
                    wn = nat_pool.tile([P, D], F32, tag="wnat", name="wnat")
                    nc.sync.dma_start(out=wn, in_=wsrc[er * P:(er + 1) * P, :])
                    wnh = nat_pool.tile([P, D], FP16, tag="wnh", name="wnh")
                    nc.vector.tensor_copy(out=wnh, in_=wn)
                    pt = ppool_t.tile([P, 4 * P], F32, tag="tp", name="tp")
                    pth = pt.bitcast(FP16)
                    for dc in range(DC):
                        nc.tensor.transpose(
                            pth[:, dc * P:(dc + 1) * P],
                            wnh[:, dc * P:(dc + 1) * P], ident_bf)
                    nc.vector.tensor_copy(
                        out=wdst[:, :, er * P:(er + 1) * P],
                        in_=pth.rearrange("p (a b) -> p a b", a=DC))
            for dc in range(DC):                  # w0 [D, E] -> w0T (bf16)
                wn = nat_pool.tile([P, E], F32, tag="w0nat", name="w0nat")
                nc.sync.dma_start(out=wn, in_=w0[dc * P:(dc + 1) * P, :])
                for ec in range(E // P):
                    pt = ppool_t.tile([P, P], F32, tag="tp", name="tp")
                    nc.tensor.transpose(pt, wn[:, ec * P:(ec + 1) * P],
                                        identity)
                    nc.vector.tensor_copy(
                        out=w0T[:, ec, dc * P:(dc + 1) * P], in_=pt)

            # -- activations: per s-group of 512: transpose + project --
            for t in range(SG):
                s0 = t * SGW
                for src, kind in ((q, "q"), (k, "k"), (v, "v")):
                    xtg = xtg_pool.tile([P, DC, SGW], FP16, tag="xtg",
                                        name="xtg")
                    for sc4 in range(SGW // P):
                        xn = nat_pool.tile([P, D], F32, tag="xnat",
                                           name="xnat")
                        nc.sync.dma_start(
                            out=xn,
                            in_=src[s0 + sc4 * P:s0 + (sc4 + 1) * P, :])
                        # fp16-convert once, then 8 fp16 transposes (1 cyc/
                        # row) into ONE PSUM bank and a single strided copy;
                        # alternate engines for converts and copies
                        xnh = nat_pool.tile([P, D], FP16, tag="xnh",
                                            name="xnh")
                        if sc4 % 2 == 0:
                            nc.vector.tensor_copy(out=xnh, in_=xn)
                        else:
                            nc.scalar.copy(out=xnh, in_=xn)
                        pt = ppool_t.tile([P, 4 * P], F32, tag="tp",
                                          name="tp")
                        pth = pt.bitcast(FP16)
                        for dc in range(DC):
                            nc.tensor.transpose(
                                pth[:, dc * P:(dc + 1) * P],
                                xnh[:, dc * P:(dc + 1) * P], ident_bf)
                        dst8 = xtg[:, :, sc4 * P:(sc4 + 1) * P]
                        pt8 = pth.rearrange("p (a b) -> p a b", a=DC)
                        if sc4 % 2 == 0:
                            nc.scalar.copy(out=dst8, in_=pt8)
                        else:
                            nc.vector.tensor_copy(out=dst8, in_=pt8)
                    if kind in ("q", "k"):
                        wT = wqT if kind == "q" else wkT
                        dst = qpT if kind == "q" else kpT
                        for et in range(2):
                            acc = ppool_a.tile([P, SGW], F32, tag="acc",
                                               name="acc")
                            for dc in range(DC):
                                nc.tensor.matmul(
                                    acc,
                                    wT[:, dc, et * P:(et + 1) * P],
                                    xtg[:, dc, :],
                                    start=(dc == 0), stop=(dc == DC - 1))
                            if et == 0:
                                nc.vector.tensor_copy(
                                    out=dst[et][:, s0:s0 + SGW], in_=acc)
                            else:
                                nc.scalar.copy(
                                    out=dst[et][:, s0:s0 + SGW], in_=acc)
                    else:
                        for sc4 in range(SGW // P):
                            scg = t * (SGW // P) + sc4
                            accv = ppool_a.tile([P, E], F32, tag="acc",
                                                name="acc",
                                                padded_shape=[P, SGW])
                            for dc in range(DC):
                                nc.tensor.matmul(
                                    accv,
                                    xtg[:, dc, sc4 * P:(sc4 + 1) * P],
                                    wvT[:, dc, :],
                                    start=(dc == 0), stop=(dc == DC - 1))
                            for h in range(HG):
                                nc.vector.tensor_copy(
                                    out=vps[:, scg, h * VPAD:h * VPAD + DK],
                                    in_=accv[:, h * DK:(h + 1) * DK])

        # ========= phase A + N + W, interleaved per q-block of 512 =========
        with tc.tile_pool(name="a_ps", bufs=1, space="PSUM") as psum, \
             tc.tile_pool(name="a_sb", bufs=1) as asb:
            for qb in range(NQB):
                q0 = qb * QB
                x65s = {}
                for et in range(2):
                    hA, hB = 2 * et, 2 * et + 1
                    xaccA = psum.tile([P, QB], F32, tag="xacc", bufs=2,
                                      name="xaccA")
                    xaccB = psum.tile([P, QB], F32, tag="xacc", bufs=2,
                                      name="xaccB")
                    def emit_pv(kkp, ab):
                        nc.tensor.matmul(
                            xaccA[:VW, :],
                            vps[:, kkp, hA * VPAD:hA * VPAD + VW],
                            ab[0],
                            start=(kkp == 0), stop=(kkp == NST - 1))
                        nc.tensor.matmul(
                            xaccB[:VW, :],
                            vps[:, kkp, hB * VPAD:hB * VPAD + VW],
                            ab[1],
                            start=(kkp == 0), stop=(kkp == NST - 1))

                    prev = None
                    for kk in range(NST):
                        stA = psum.tile([P, QB], F32, tag="st", bufs=6,
                                        name="stA")
                        stB = psum.tile([P, QB], F32, tag="st", bufs=6,
                                        name="stB")
                        nc.tensor.matmul(
                            stA,
                            kpT[et][0:DK, kk * P:(kk + 1) * P],
                            qpT[et][0:DK, q0:q0 + QB],
                            start=True, stop=True, tile_position=(0, 0))
                        nc.tensor.matmul(
                            stB,
                            kpT[et][DK:P, kk * P:(kk + 1) * P],
                            qpT[et][DK:P, q0:q0 + QB],
                            start=True, stop=True, tile_position=(DK, 0))
                        attA = asb.tile([P, QB], FP16, tag="att", bufs=6,
                                        name="attA")
                        attB = asb.tile([P, QB], FP16, tag="att", bufs=6,
                                        name="attB")
                        nc.scalar.activation(attA, stA, EXPF, scale=0.125)
                        if kk % 3 == 0:
                            # exact exp for head B every 3rd stripe: keeps the
                            # Schraudolph share at 1/3, ACT load ~= PE load
                            nc.scalar.activation(attB, stB, EXPF, scale=0.125)
                        else:
                            nc.vector.tensor_scalar(
                                out=attB.bitcast(I16), in0=stB,
                                scalar1=SCH_A, scalar2=SCH_B,
                                op0=mybir.AluOpType.mult,
                                op1=mybir.AluOpType.add)
                        # software pipeline: PV for stripe kk-1 issues behind
                        # this stripe's scores so the PE never waits on exp
                        if prev is not None:
                            emit_pv(kk - 1, prev)
                        prev = (attA, attB)
                    emit_pv(NST - 1, prev)
                    x65A = asb.tile([VW, QB], F32, tag="x65", bufs=8,
                                    name="x65A")
                    x65B = asb.tile([VW, QB], F32, tag="x65", bufs=8,
                                    name="x65B")
                    nc.scalar.copy(out=x65A, in_=xaccA[:VW, :])
                    nc.vector.tensor_copy(out=x65B, in_=xaccB[:VW, :])
                    x65s[hA], x65s[hB] = x65A, x65B

                # ---- phase N + W, two-stage pipeline over q-chunks so
                # stage-2 PE work (transpose-back + w0 matmuls) fills the
                # DVE-latency gaps of stage-1 (transpose -> 1/rowsum) ----
                def nw_stage1(qc):
                    xs2s = []
                    for et in range(2):
                        xs2 = asb.tile([P, 2 * DK], FP16, tag="xs2", bufs=6,
                                       name="xs2")
                        for hp in range(2):
                            h = 2 * et + hp
                            tp = psum.tile([P, QB], F32, tag="st", bufs=6,
                                           name="ntp")
                            nc.tensor.transpose(
                                tp[:, :VW],
                                x65s[h][:, qc * P:(qc + 1) * P],
                                identity[:VW, :VW])
                            rcp = asb.tile([P, 1], F32, tag="rcp", bufs=8,
                                           name="rcp")
                            nc.vector.reciprocal(rcp, tp[:, DK:DK + 1])
                            nc.vector.tensor_scalar_mul(
                                xs2[:, hp * DK:(hp + 1) * DK],
                                tp[:, 0:DK], rcp)
                        xs2s.append(xs2)
                    return xs2s

                def nw_stage2(qc, xs2s):
                    xwc = []
                    for et in range(2):
                        tb = psum.tile([P, QB], F32, tag="st", bufs=6,
                                       name="ntb")
                        tbb = tb.bitcast(FP16)
                        nc.tensor.transpose(tbb[:, :P], xs2s[et], ident_bf)
                        xwt = asb.tile([P, P], FP16, tag="xw", bufs=4,
                                       name="xw")
                        nc.vector.tensor_copy(out=xwt, in_=tbb[:, :P])
                        xwc.append(xwt)
                    oacc = [psum.tile([P, 512], F32, tag="st", bufs=6,
                                      name=f"oacc{j}") for j in range(2)]
                    for et in range(2):
                        for j in range(2):
                            nc.tensor.matmul(
                                oacc[j],
                                xwc[et],
                                w0T[:, et, j * 512:(j + 1) * 512],
                                start=(et == 0), stop=(et == 1))
                    osb = asb.tile([P, D], F32, tag="osb", bufs=3, name="osb")
                    nc.vector.tensor_copy(out=osb[:, 0:512], in_=oacc[0])
                    nc.scalar.copy(out=osb[:, 512:1024], in_=oacc[1])
                    nc.sync.dma_start(
                        out=out[q0 + qc * P:q0 + (qc + 1) * P, :], in_=osb)

                pend = []
                for qc in range(QB // P):
                    pend.append((qc, nw_stage1(qc)))
                    if qc >= 1:
                        nw_stage2(*pend.pop(0))
                while pend:
                    nw_stage2(*pend.pop(0))


def _enable_ldw_opt():
    """Compile this kernel with walrus's LDWEIGHTS background-buffer
    optimization (off by default in this path): stationary loads then
    overlap in-flight matmuls instead of serializing behind them."""
    if getattr(bass_utils.run_command, "_ldw_patched", False):
        return
    orig = bass_utils.run_command

    def run_command_ldw(argv, **kw):
        argv = ["--enable-ldw-opt=true" if a == "--enable-ldw-opt=false"
                else a for a in argv]
        return orig(argv, **kw)

    run_command_ldw._ldw_patched = True
    bass_utils.run_command = run_command_ldw


def build_program():
    nc = bacc.Bacc("TRN2", target_bir_lowering=False, debug=False,
                   num_devices=NCORES)
    q = nc.dram_tensor("q", (S, D), F32, kind="ExternalInput").ap()
    k = nc.dram_tensor("k", (S, D), F32, kind="ExternalInput").ap()
    v = nc.dram_tensor("v", (S, D), F32, kind="ExternalInput").ap()
    wq = nc.dram_tensor("wq", (E, D), F32, kind="ExternalInput").ap()
    wk = nc.dram_tensor("wk", (E, D), F32, kind="ExternalInput").ap()
    wv = nc.dram_tensor("wv", (E, D), F32, kind="ExternalInput").ap()
    w0 = nc.dram_tensor("w0", (D, E), F32, kind="ExternalInput").ap()
    out = nc.dram_tensor("out", (S, D), F32, kind="ExternalOutput").ap()
    with tile.TileContext(nc) as tc:
        kernel_body(tc, q, k, v, wq, wk, wv, w0, out)
    nc.compile()
    return nc


_NC_CACHE = None


def _get_program():
    global _NC_CACHE
    if _NC_CACHE is None:
        _NC_CACHE = build_program()
    return _NC_CACHE


def make_in_maps(q, k, v, wq, wk, wv, w0):
    arrs = [np.asarray(a, dtype=np.float32)
            for a in (q, k, v, wq, wk, wv, w0)]
    q, k, v, wq, wk, wv, w0 = arrs
    in_maps = []
    for c in range(NCORES):
        b, g = c // GROUPS, c % GROUPS
        e0 = g * E
        in_maps.append({
            "q": np.ascontiguousarray(q[b]),
            "k": np.ascontiguousarray(k[b]),
            "v": np.ascontiguousarray(v[b]),
            "wq": np.ascontiguousarray(wq[e0:e0 + E, :]),
            "wk": np.ascontiguousarray(wk[e0:e0 + E, :]),
            "wv": np.ascontiguousarray(wv[e0:e0 + E, :]),
            "w0": np.ascontiguousarray(w0[:, e0:e0 + E]),
        })
    return in_maps


def gather_out(results):
    out = np.zeros((B, S, D), dtype=np.float32)
    for c in range(NCORES):
        b = c // GROUPS
        out[b] += results[c]["out"]
    return out


def _install_ntff_hook_shim():
    """This image's antenv lacks axon_hooks; recreate it so trace=True works.

    Mirrors trn_agent_boot.trn_boot._ntff_profile_via_ctypes against
    /opt/axon/libaxon_pjrt.so.
    """
    import sys, types, ctypes, contextlib
    if "antenv.axon_hooks" in sys.modules:
        return
    mod = types.ModuleType("antenv.axon_hooks")
    mod._hook = None

    def set_axon_ntff_profile_hook(h):
        mod._hook = h

    def get_axon_ntff_profile_hook():
        return mod._hook

    mod.set_axon_ntff_profile_hook = set_axon_ntff_profile_hook
    mod.get_axon_ntff_profile_hook = get_axon_ntff_profile_hook
    sys.modules["antenv.axon_hooks"] = mod
    try:
        import antenv
        antenv.axon_hooks = mod
    except ImportError:
        pass

    so_path = "/opt/axon/libaxon_pjrt.so"
    try:
        lib = ctypes.CDLL(so_path)
        if not hasattr(lib, "axon_start_nrt_profile"):
            return
        lib.axon_start_nrt_profile.argtypes = [
            ctypes.POINTER(ctypes.c_int64), ctypes.c_size_t]
        lib.axon_start_nrt_profile.restype = ctypes.c_int64
        lib.axon_stop_nrt_profile.argtypes = [ctypes.c_char_p]
        lib.axon_stop_nrt_profile.restype = ctypes.c_int64
    except OSError:
        return

    @contextlib.contextmanager
    def _hook(output_dir, device_ids):
        import jax
        jax.devices()
        if device_ids:
            ids = (ctypes.c_int64 * len(device_ids))(*device_ids)
            rc = lib.axon_start_nrt_profile(ids, len(device_ids))
        else:
            rc = lib.axon_start_nrt_profile(None, 0)
        if rc != 0:
            raise RuntimeError(f"axon_start_nrt_profile rc={rc}")
        try:
            yield
        finally:
            n = lib.axon_stop_nrt_profile(str(output_dir).encode())
            print(f"profile: {n} file(s) written to {output_dir}")

    mod._hook = _hook


def kernel(q, k, v, wq, wk, wv, w0, _trace=False, _tmpdir=None):
    if _trace:
        _install_ntff_hook_shim()
    nc = _get_program()
    in_maps = make_in_maps(q, k, v, wq, wk, wv, w0)
    res = bass_utils.run_bass_kernel_spmd(
        nc, in_maps, core_ids=list(range(NCORES)),
        trace=_trace, tmpdir=_tmpdir)
    out = gather_out(res.results)
    if _trace:
        return out, res
    return out


# revision 18
# speedup vs baseline: 1.1752x; 1.0733x over previous
"""Multi-head attention (B=2, S=4096, D=1024, H=16) on 8 NeuronCores.

Sharding: core c = (batch b = c // 4, head-group g = c % 4).  Each head-group
owns 4 heads = 256 projection features.  Per core:
  - PE-transpose q/k/v[b] tiles into [d, s] layout; projections with
    row-sliced wq/wk/wv; qpT/kpT stored bf16, vp stored fp32.
  - attention per head-PAIR: scores for heads (2et, 2et+1) computed as two
    row-tiled bf16 matmuls (K=64 each) running concurrently on the PE array
    (tile_position (0,0) / (64,0)).  exp for head A on the ACT engine
    (exact), head B on the DVE via the Schraudolph bit-trick (one
    tensor_scalar, int32 out, bitcast back to f32).  Softmax normalization
    deferred via a ones column in the PV stationary operand.
  - per q-block of 512: normalize (transpose -> 1/rowsum -> transpose back,
    bf16) and project with w0 immediately, so output DMA overlaps attention.
Host sums the 4 partials per batch.
"""

import numpy as np
from contextlib import ExitStack

import concourse.bass as bass
import concourse.bacc as bacc
import concourse.tile as tile
from concourse import mybir, bass_utils
from concourse.masks import make_identity

B, S, D, H = 2, 4096, 1024, 16
DK = D // H          # 64
NCORES = 8
GROUPS = 4           # head-groups (tensor parallel)
HG = H // GROUPS     # 4 heads per group
E = HG * DK          # 256 features per group

F32 = mybir.dt.float32
F32R = mybir.dt.float32r
FP16 = mybir.dt.float16
I16 = mybir.dt.int16

P = 128              # partitions
SC = S // P          # 32 s-chunks of 128
SG = 8               # s-groups in phase T
SGW = S // SG        # 512
DC = D // P          # 8 d-chunks
QB = 512             # q-block in attention
NQB = S // QB        # 8
NST = SC             # 32 k-stripes of 128
VW = DK + 1          # vp columns per head incl. ones column (65)
VPAD = 66            # padded per-head stride in vps tile

# Schraudolph fast-exp in fp16: exp(0.125*x) ~= bitcast_fp16(int16(A*x + B))
# A = 2^10/ln2 * 0.125, B = 15*2^10 - 59.3 (min max-rel-err shift)
SCH_A = (1024.0 / 0.6931471805599453) * 0.125
SCH_B = 15300.7

EXPF = mybir.ActivationFunctionType.Exp


def _r(ap):
    return ap.bitcast(F32R)


def kernel_body(tc, q, k, v, wq, wk, wv, w0, out):
    nc = tc.nc
    ctx = ExitStack()
    with ctx:
        ident_pool = ctx.enter_context(tc.tile_pool(name="ident", bufs=1))
        identity = ident_pool.tile([P, P], F32, tag="ident", name="identity")
        make_identity(nc, identity)
        ident_bf = ident_pool.tile([P, P], FP16, tag="identbf",
                                   name="ident_bf")
        nc.vector.tensor_copy(out=ident_bf, in_=identity)

        # persistent across T..W
        w0T_pool = ctx.enter_context(tc.tile_pool(name="w0T", bufs=1))
        w0T = w0T_pool.tile([P, 2, D], FP16, tag="w0T", name="w0T")

        # persistent through phase A
        proj_pool = ctx.enter_context(tc.tile_pool(name="proj", bufs=1))
        qpT = [proj_pool.tile([P, S], FP16, tag=f"qpT{i}", name=f"qpT{i}")
               for i in range(2)]
        kpT = [proj_pool.tile([P, S], FP16, tag=f"kpT{i}", name=f"kpT{i}")
               for i in range(2)]
        vps = proj_pool.tile([P, SC, HG * VPAD], FP16, tag="vps", name="vps")
        # prefill so the per-head ones columns (offset DK in each VPAD
        # stride) are already 1.0; data copies below overwrite cols 0..DK-1
        nc.vector.memset(vps, 1.0)

        # ================= phase T: transposes + projections =================
        with tc.tile_pool(name="t_wT", bufs=1) as wT_pool, \
             tc.tile_pool(name="t_nat", bufs=5) as nat_pool, \
             tc.tile_pool(name="t_xtg", bufs=3) as xtg_pool, \
             tc.tile_pool(name="t_psum", bufs=4, space="PSUM") as ppool_t, \
             tc.tile_pool(name="t_pacc", bufs=4, space="PSUM") as ppool_a:

            wqT = wT_pool.tile([P, DC, E], FP16, tag="wqT", name="wqT")
            wkT = wT_pool.tile([P, DC, E], FP16, tag="wkT", name="wkT")
            wvT = wT_pool.tile([P, DC, E], FP16, tag="wvT", name="wvT")

            # -- weights: load natural, fp16-convert, PE-transpose (1 cyc/
            # row) 8 tiles into one PSUM bank, single strided copy out --
            for wsrc, wdst in ((wq, wqT), (wk, wkT), (wv, wvT)):
                for er in range(E // P):          # 2 chunks of output rows
                    wn = nat_pool.tile([P, D], F32, tag="wnat", name="wnat")
                    nc.sync.dma_start(out=wn, in_=wsrc[er * P:(er + 1) * P, :])
                    wnh = nat_pool.tile([P, D], FP16, tag="wnh", name="wnh")
                    nc.vector.tensor_copy(out=wnh, in_=wn)
                    pt = ppool_t.tile([P, 4 * P], F32, tag="tp", name="tp")
                    pth = pt.bitcast(FP16)
                    for dc in range(DC):
                        nc.tensor.transpose(
                            pth[:, dc * P:(dc + 1) * P],
                            wnh[:, dc * P:(dc + 1) * P], ident_bf)
                    nc.vector.tensor_copy(
                        out=wdst[:, :, er * P:(er + 1) * P],
                        in_=pth.rearrange("p (a b) -> p a b", a=DC))
            for dc in range(DC):                  # w0 [D, E] -> w0T (bf16)
                wn = nat_pool.tile([P, E], F32, tag="w0nat", name="w0nat")
                nc.sync.dma_start(out=wn, in_=w0[dc * P:(dc + 1) * P, :])
                for ec in range(E // P):
                    pt = ppool_t.tile([P, P], F32, tag="tp", name="tp")
                    nc.tensor.transpose(pt, wn[:, ec * P:(ec + 1) * P],
                                        identity)
                    nc.vector.tensor_copy(
                        out=w0T[:, ec, dc * P:(dc + 1) * P], in_=pt)

            # -- activations: per s-group of 512: transpose + project --
            for t in range(SG):
                s0 = t * SGW
                for src, kind in ((q, "q"), (k, "k"), (v, "v")):
                    xtg = xtg_pool.tile([P, DC, SGW], FP16, tag="xtg",
                                        name="xtg")
                    for sc4 in range(SGW // P):
                        xn = nat_pool.tile([P, D], F32, tag="xnat",
                                           name="xnat")
                        nc.sync.dma_start(
                            out=xn,
                            in_=src[s0 + sc4 * P:s0 + (sc4 + 1) * P, :])
                        # fp16-convert once, then 8 fp16 transposes (1 cyc/
                        # row) into ONE PSUM bank and a single strided copy;
                        # alternate engines for converts and copies
                        xnh = nat_pool.tile([P, D], FP16, tag="xnh",
                                            name="xnh")
                        if sc4 % 2 == 0:
                            nc.vector.tensor_copy(out=xnh, in_=xn)
                        else:
                            nc.scalar.copy(out=xnh, in_=xn)
                        pt = ppool_t.tile([P, 4 * P], F32, tag="tp",
                                          name="tp")
                        pth = pt.bitcast(FP16)
                        for dc in range(DC):
                            nc.tensor.transpose(
                                pth[:, dc * P:(dc + 1) * P],
                                xnh[:, dc * P:(dc + 1) * P], ident_bf)
                        dst8 = xtg[:, :, sc4 * P:(sc4 + 1) * P]
                        pt8 = pth.rearrange("p (a b) -> p a b", a=DC)
                        if sc4 % 2 == 0:
                            nc.scalar.copy(out=dst8, in_=pt8)
                        else:
                            nc.vector.tensor_copy(out=dst8, in_=pt8)
                    if kind in ("q", "k"):
                        wT = wqT if kind == "q" else wkT
                        dst = qpT if kind == "q" else kpT
                        for et in range(2):
                            acc = ppool_a.tile([P, SGW], F32, tag="acc",
                                               name="acc")
                            for dc in range(DC):
                                nc.tensor.matmul(
                                    acc,
                                    wT[:, dc, et * P:(et + 1) * P],
                                    xtg[:, dc, :],
                                    start=(dc == 0), stop=(dc == DC - 1))
                            if et == 0:
                                nc.vector.tensor_copy(
                                    out=dst[et][:, s0:s0 + SGW], in_=acc)
                            else:
                                nc.scalar.copy(
                                    out=dst[et][:, s0:s0 + SGW], in_=acc)
                    else:
                        for sc4 in range(SGW // P):
                            scg = t * (SGW // P) + sc4
                            accv = ppool_a.tile([P, E], F32, tag="acc",
                                                name="acc",
                                                padded_shape=[P, SGW])
                            for dc in range(DC):
                                nc.tensor.matmul(
                                    accv,
                                    xtg[:, dc, sc4 * P:(sc4 + 1) * P],
                                    wvT[:, dc, :],
                                    start=(dc == 0), stop=(dc == DC - 1))
                            for h in range(HG):
                                nc.vector.tensor_copy(
                                    out=vps[:, scg, h * VPAD:h * VPAD + DK],
                                    in_=accv[:, h * DK:(h + 1) * DK])

        # ========= phase A + N + W, interleaved per q-block of 512 =========
        with tc.tile_pool(name="a_ps", bufs=1, space="PSUM") as psum, \
             tc.tile_pool(name="a_sb", bufs=1) as asb:
            for qb in range(NQB):
                q0 = qb * QB
                x65s = {}
                for et in range(2):
                    hA, hB = 2 * et, 2 * et + 1
                    xaccA = psum.tile([P, QB], F32, tag="xacc", bufs=2,
                                      name="xaccA")
                    xaccB = psum.tile([P, QB], F32, tag="xacc", bufs=2,
                                      name="xaccB")
                    def emit_pv(kkp, ab):
                        nc.tensor.matmul(
                            xaccA[:VW, :],
                            vps[:, kkp, hA * VPAD:hA * VPAD + VW],
                            ab[0],
                            start=(kkp == 0), stop=(kkp == NST - 1))
                        nc.tensor.matmul(
                            xaccB[:VW, :],
                            vps[:, kkp, hB * VPAD:hB * VPAD + VW],
                            ab[1],
                            start=(kkp == 0), stop=(kkp == NST - 1))

                    prev = None
                    for kk in range(NST):
                        stA = psum.tile([P, QB], F32, tag="st", bufs=6,
                                        name="stA")
                        stB = psum.tile([P, QB], F32, tag="st", bufs=6,
                                        name="stB")
                        nc.tensor.matmul(
                            stA,
                            kpT[et][0:DK, kk * P:(kk + 1) * P],
                            qpT[et][0:DK, q0:q0 + QB],
                            start=True, stop=True, tile_position=(0, 0))
                        nc.tensor.matmul(
                            stB,
                            kpT[et][DK:P, kk * P:(kk + 1) * P],
                            qpT[et][DK:P, q0:q0 + QB],
                            start=True, stop=True, tile_position=(DK, 0))
                        attA = asb.tile([P, QB], FP16, tag="att", bufs=6,
                                        name="attA")
                        attB = asb.tile([P, QB], FP16, tag="att", bufs=6,
                                        name="attB")
                        nc.scalar.activation(attA, stA, EXPF, scale=0.125)
                        if kk % 3 == 0:
                            # exact exp for head B every 3rd stripe: keeps the
                            # Schraudolph share at 1/3, ACT load ~= PE load
                            nc.scalar.activation(attB, stB, EXPF, scale=0.125)
                        else:
                            nc.vector.tensor_scalar(
                                out=attB.bitcast(I16), in0=stB,
                                scalar1=SCH_A, scalar2=SCH_B,
                                op0=mybir.AluOpType.mult,
                                op1=mybir.AluOpType.add)
                        # software pipeline: PV for stripe kk-1 issues behind
                        # this stripe's scores so the PE never waits on exp
                        if prev is not None:
                            emit_pv(kk - 1, prev)
                        prev = (attA, attB)
                    emit_pv(NST - 1, prev)
                    x65A = asb.tile([VW, QB], F32, tag="x65", bufs=8,
                                    name="x65A")
                    x65B = asb.tile([VW, QB], F32, tag="x65", bufs=8,
                                    name="x65B")
                    nc.vector.tensor_copy(out=x65A, in_=xaccA[:VW, :])
                    nc.vector.tensor_copy(out=x65B, in_=xaccB[:VW, :])
                    x65s[hA], x65s[hB] = x65A, x65B

                # ---- phase N + W, two-stage pipeline over q-chunks so
                # stage-2 PE work (transpose-back + w0 matmuls) fills the
                # DVE-latency gaps of stage-1 (transpose -> 1/rowsum) ----
                def nw_stage1(qc):
                    xs2s = []
                    for et in range(2):
                        xs2 = asb.tile([P, 2 * DK], FP16, tag="xs2", bufs=6,
                                       name="xs2")
                        for hp in range(2):
                            h = 2 * et + hp
                            tp = psum.tile([P, QB], F32, tag="st", bufs=6,
                                           name="ntp")
                            nc.tensor.transpose(
                                tp[:, :VW],
                                x65s[h][:, qc * P:(qc + 1) * P],
                                identity[:VW, :VW])
                            rcp = asb.tile([P, 1], F32, tag="rcp", bufs=8,
                                           name="rcp")
                            nc.vector.reciprocal(rcp, tp[:, DK:DK + 1])
                            nc.vector.tensor_scalar_mul(
                                xs2[:, hp * DK:(hp + 1) * DK],
                                tp[:, 0:DK], rcp)
                        xs2s.append(xs2)
                    return xs2s

                def nw_stage2(qc, xs2s):
                    xwc = []
                    for et in range(2):
                        tb = psum.tile([P, QB], F32, tag="st", bufs=6,
                                       name="ntb")
                        tbb = tb.bitcast(FP16)
                        nc.tensor.transpose(tbb[:, :P], xs2s[et], ident_bf)
                        xwt = asb.tile([P, P], FP16, tag="xw", bufs=4,
                                       name="xw")
                        nc.vector.tensor_copy(out=xwt, in_=tbb[:, :P])
                        xwc.append(xwt)
                    oacc = [psum.tile([P, 512], F32, tag="st", bufs=6,
                                      name=f"oacc{j}") for j in range(2)]
                    for et in range(2):
                        for j in range(2):
                            nc.tensor.matmul(
                                oacc[j],
                                xwc[et],
                                w0T[:, et, j * 512:(j + 1) * 512],
                                start=(et == 0), stop=(et == 1))
                    osb = asb.tile([P, D], F32, tag="osb", bufs=3, name="osb")
                    nc.vector.tensor_copy(out=osb[:, 0:512], in_=oacc[0])
                    nc.vector.tensor_copy(out=osb[:, 512:1024], in_=oacc[1])
                    nc.sync.dma_start(
                        out=out[q0 + qc * P:q0 + (qc + 1) * P, :], in_=osb)

                pend = []
                for qc in range(QB // P):
                    pend.append((qc, nw_stage1(qc)))
                    if qc >= 1:
                        nw_stage2(*pend.pop(0))
                while pend:
                    nw_stage2(*pend.pop(0))


def _enable_ldw_opt():
    """Compile this kernel with walrus's LDWEIGHTS background-buffer
    optimization (off by default in this path): stationary loads then
    overlap in-flight matmuls instead of serializing behind them."""
    if getattr(bass_utils.run_command, "_ldw_patched", False):
        return
    orig = bass_utils.run_command

    def run_command_ldw(argv, **kw):
        argv = ["--enable-ldw-opt=true" if a == "--enable-ldw-opt=false"
                else a for a in argv]
        return orig(argv, **kw)

    run_command_ldw._ldw_patched = True
    bass_utils.run_command = run_command_ldw


def build_program():
    nc = bacc.Bacc("TRN2", target_bir_lowering=False, debug=False,
                   num_devices=NCORES)
    q = nc.dram_tensor("q", (S, D), F32, kind="ExternalInput").ap()
    k = nc.dram_tensor("k", (S, D), F32, kind="ExternalInput").ap()
    v = nc.dram_tensor("v", (S, D), F32, kind="ExternalInput").ap()
    wq = nc.dram_tensor("wq", (E, D), F32, kind="ExternalInput").ap()
    wk = nc.dram_tensor("wk", (E, D), F32, kind="ExternalInput").ap()
    wv = nc.dram_tensor("wv", (E, D), F32, kind="ExternalInput").ap()
    w0 = nc.dram_tensor("w0", (D, E), F32, kind="ExternalInput").ap()
    out = nc.dram_tensor("out", (S, D), F32, kind="ExternalOutput").ap()
    with tile.TileContext(nc) as tc:
        kernel_body(tc, q, k, v, wq, wk, wv, w0, out)
    nc.compile()
    return nc


_NC_CACHE = None


def _get_program():
    global _NC_CACHE
    if _NC_CACHE is None:
        _NC_CACHE = build_program()
    return _NC_CACHE


def make_in_maps(q, k, v, wq, wk, wv, w0):
    arrs = [np.asarray(a, dtype=np.float32)
            for a in (q, k, v, wq, wk, wv, w0)]
    q, k, v, wq, wk, wv, w0 = arrs
    in_maps = []
    for c in range(NCORES):
        b, g = c // GROUPS, c % GROUPS
        e0 = g * E
        in_maps.append({
            "q": np.ascontiguousarray(q[b]),
            "k": np.ascontiguousarray(k[b]),
            "v": np.ascontiguousarray(v[b]),
            "wq": np.ascontiguousarray(wq[e0:e0 + E, :]),
            "wk": np.ascontiguousarray(wk[e0:e0 + E, :]),
            "wv": np.ascontiguousarray(wv[e0:e0 + E, :]),
            "w0": np.ascontiguousarray(w0[:, e0:e0 + E]),
        })
    return in_maps


def gather_out(results):
    out = np.zeros((B, S, D), dtype=np.float32)
    for c in range(NCORES):
        b = c // GROUPS
        out[b] += results[c]["out"]
    return out


def _install_ntff_hook_shim():
    """This image's antenv lacks axon_hooks; recreate it so trace=True works.

    Mirrors trn_agent_boot.trn_boot._ntff_profile_via_ctypes against
    /opt/axon/libaxon_pjrt.so.
    """
    import sys, types, ctypes, contextlib
    if "antenv.axon_hooks" in sys.modules:
        return
    mod = types.ModuleType("antenv.axon_hooks")
    mod._hook = None

    def set_axon_ntff_profile_hook(h):
        mod._hook = h

    def get_axon_ntff_profile_hook():
        return mod._hook

    mod.set_axon_ntff_profile_hook = set_axon_ntff_profile_hook
    mod.get_axon_ntff_profile_hook = get_axon_ntff_profile_hook
    sys.modules["antenv.axon_hooks"] = mod
    try:
        import antenv
        antenv.axon_hooks = mod
    except ImportError:
        pass

    so_path = "/opt/axon/libaxon_pjrt.so"
    try:
        lib = ctypes.CDLL(so_path)
        if not hasattr(lib, "axon_start_nrt_profile"):
            return
        lib.axon_start_nrt_profile.argtypes = [
            ctypes.POINTER(ctypes.c_int64), ctypes.c_size_t]
        lib.axon_start_nrt_profile.restype = ctypes.c_int64
        lib.axon_stop_nrt_profile.argtypes = [ctypes.c_char_p]
        lib.axon_stop_nrt_profile.restype = ctypes.c_int64
    except OSError:
        return

    @contextlib.contextmanager
    def _hook(output_dir, device_ids):
        import jax
        jax.devices()
        if device_ids:
            ids = (ctypes.c_int64 * len(device_ids))(*device_ids)
            rc = lib.axon_start_nrt_profile(ids, len(device_ids))
        else:
            rc = lib.axon_start_nrt_profile(None, 0)
        if rc != 0:
            raise RuntimeError(f"axon_start_nrt_profile rc={rc}")
        try:
            yield
        finally:
            n = lib.axon_stop_nrt_profile(str(output_dir).encode())
            print(f"profile: {n} file(s) written to {output_dir}")

    mod._hook = _hook


def kernel(q, k, v, wq, wk, wv, w0, _trace=False, _tmpdir=None):
    if _trace:
        _install_ntff_hook_shim()
    nc = _get_program()
    in_maps = make_in_maps(q, k, v, wq, wk, wv, w0)
    res = bass_utils.run_bass_kernel_spmd(
        nc, in_maps, core_ids=list(range(NCORES)),
        trace=_trace, tmpdir=_tmpdir)
    out = gather_out(res.results)
    if _trace:
        return out, res
    return out
